# revision 2
# baseline (speedup 1.0000x reference)
"""CrossAttention (single-head) Trainium2 kernel, 8-core data-parallel.

Full inputs in, full output out. Internally: batch 16 is sharded 2-per-core
across 8 NeuronCores; each core runs the whole attention layer for its two
batches in bf16 (f32 PSUM accumulation), with activations kept in transposed
[d, s] layout so every matmul contracts over the partition dim without any
on-chip transposes of large tensors.
"""

import sys

sys.path.insert(0, "/opt/trn_rl_repo")

import numpy as np
import ml_dtypes

import concourse.bass as bass
import concourse.mybir as mybir
import concourse.tile as tile
from concourse.bass_utils import run_bass_kernel_spmd

BF16 = mybir.dt.bfloat16
F32 = mybir.dt.float32
AF = mybir.ActivationFunctionType

N_CORES = 8
B, S, D = 16, 2048, 1024
NB = B // N_CORES          # batches per core
KC = D // 128              # 8 chunks of 128 along d
ST = S // 128              # 16 tiles of 128 along s
NBLK = S // 512            # 4 blocks of 512 along s
SCALE = 1.0 / np.sqrt(np.float32(D))  # 1/32


def _split_waits(nc, limit=1):
    """Walrus in this container allows at most one sync wait per instruction:
    hoist excess waits onto NoOp carriers inserted just before."""
    n_new = 0
    for f in nc.m.functions:
        for bb in f.blocks:
            new_insts = []
            for inst in bb.instructions:
                si = inst.sync_info
                waits = list(si.on_wait) if si and si.on_wait else []
                if len(waits) > limit:
                    excess, keep = waits[:-limit], waits[-limit:]
                    for i in range(0, len(excess), limit):
                        chunk = excess[i:i + limit]
                        nop = mybir.InstNoOp(
                            name=f"{inst.name}-ws-{n_new}",
                            ins=[], outs=[],
                            sync_info=mybir.SyncInfo(on_wait=chunk, on_update=[]),
                        )
                        nop.engine = inst.engine
                        new_insts.append(nop)
                        n_new += 1
                    si.on_wait = keep
                new_insts.append(inst)
            bb.instructions[:] = new_insts
    return n_new


def build_program():
    nc = bass.Bass()

    qT_d = nc.declare_dram_parameter("qT", [NB, D, S], BF16, isOutput=False)
    kT_d = nc.declare_dram_parameter("kT", [NB, D, S], BF16, isOutput=False)
    vT_d = nc.declare_dram_parameter("vT", [NB, D, S], BF16, isOutput=False)
    Wq_d = nc.declare_dram_parameter("Wq", [D, D], BF16, isOutput=False)
    Wk_d = nc.declare_dram_parameter("Wk", [D, D], BF16, isOutput=False)
    Wv_d = nc.declare_dram_parameter("Wv", [D, D], BF16, isOutput=False)
    Wo_d = nc.declare_dram_parameter("Wo", [D, D], BF16, isOutput=False)
    # bq pre-scaled by 1/32 and reshaped [128, KC] host-side; bk likewise unscaled
    bq_d = nc.declare_dram_parameter("bq", [128, KC], F32, isOutput=False)
    bk_d = nc.declare_dram_parameter("bk", [128, KC], F32, isOutput=False)
    bv_d = nc.declare_dram_parameter("bv", [D], F32, isOutput=False)
    bo_d = nc.declare_dram_parameter("bo", [D], F32, isOutput=False)
    out_d = nc.declare_dram_parameter("out", [NB, S, D], F32, isOutput=True)

    with tile.TileContext(nc) as tc:
        with (
            tc.tile_pool(name="w", bufs=18) as wpool,
            tc.tile_pool(name="inp", bufs=10) as inpool,
            tc.tile_pool(name="keyT", bufs=8) as kpool,
            tc.tile_pool(name="value", bufs=1) as vpool,
            tc.tile_pool(name="queryT", bufs=16) as qpool,
            tc.tile_pool(name="expT", bufs=2) as epool,
            tc.tile_pool(name="UT", bufs=2) as upool,
            tc.tile_pool(name="outb", bufs=2) as opool,
            tc.tile_pool(name="small", bufs=4) as smpool,
            tc.tile_pool(name="const", bufs=1) as cpool,
            tc.tile_pool(name="ps", bufs=6, space="PSUM") as pspool,
            tc.tile_pool(name="ps1", bufs=1, space="PSUM") as ps1pool,
            tc.tile_pool(name="psr", bufs=1, space="PSUM") as psrpool,
        ):
            # constants
            ones = cpool.tile([128, 1], BF16, tag="ones")
            nc.vector.memset(ones[:], 1.0)
            ident = cpool.tile([1, 1], F32, tag="ident")
            nc.vector.memset(ident[:], 1.0)
            bq_sb = cpool.tile([128, KC], F32, tag="bq")
            nc.sync.dma_start(out=bq_sb[:], in_=bq_d[:])
            bk_sb = cpool.tile([128, KC], F32, tag="bk")
            nc.sync.dma_start(out=bk_sb[:], in_=bk_d[:])
            bv_sb = cpool.tile([128, D], F32, tag="bv")
            ap = bv_d[:]
            nc.sync.dma_start(
                out=bv_sb[:],
                in_=bass.AP(tensor=ap.tensor, offset=ap.offset, ap=[[0, 128]] + ap.ap),
            )
            bo_sb = cpool.tile([128, D], F32, tag="bo")
            ap = bo_d[:]
            nc.sync.dma_start(
                out=bo_sb[:],
                in_=bass.AP(tensor=ap.tensor, offset=ap.offset, ap=[[0, 128]] + ap.ap),
            )

            def load_w(w_d):
                tiles = []
                for i in range(KC):
                    t = wpool.tile([128, D], BF16, tag="w", name=f"w{i}")
                    nc.sync.dma_start(out=t[:], in_=w_d[i * 128:(i + 1) * 128, :])
                    tiles.append(t)
                return tiles

            for b in range(NB):
                # ---------------- keyT[d, s] = Wk.T @ kT (+bk) ----------------
                Wk_t = load_w(Wk_d)
                keyT = [kpool.tile([128, S], BF16, tag="keyT", name=f"keyT{i}") for i in range(KC)]
                for s in range(NBLK):
                    kin = []
                    for i in range(KC):
                        t = inpool.tile([128, 512], BF16, tag="inp", name=f"in{i}")
                        nc.sync.dma_start(
                            out=t[:],
                            in_=kT_d[b, i * 128:(i + 1) * 128, s * 512:(s + 1) * 512],
                        )
                        kin.append(t)
                    for do in range(KC):
                        psum = pspool.tile([128, 512], F32, tag="ps")
                        for i in range(KC):
                            nc.tensor.matmul(
                                psum[:], Wk_t[i][:, do * 128:(do + 1) * 128], kin[i][:],
                                start=(i == 0), stop=(i == KC - 1),
                            )
                        nc.scalar.activation(
                            keyT[do][:, s * 512:(s + 1) * 512], psum[:],
                            AF.Identity, bias=bk_sb[:, do:do + 1], scale=1.0,
                        )

                # ---------------- value[s, d] = vT.T @ Wv (+bv) ----------------
                Wv_t = load_w(Wv_d)
                val = vpool.tile([128, ST, D], BF16, tag="value")
                for s in range(NBLK):
                    vin = []
                    for i in range(KC):
                        t = inpool.tile([128, 512], BF16, tag="inp", name=f"in{i}")
                        nc.sync.dma_start(
                            out=t[:],
                            in_=vT_d[b, i * 128:(i + 1) * 128, s * 512:(s + 1) * 512],
                        )
                        vin.append(t)
                    for tt in range(4):
                        t16 = s * 4 + tt
                        for n in range(2):
                            psum = pspool.tile([128, 512], F32, tag="ps")
                            for i in range(KC):
                                nc.tensor.matmul(
                                    psum[:],
                                    vin[i][:, tt * 128:(tt + 1) * 128],
                                    Wv_t[i][:, n * 512:(n + 1) * 512],
                                    start=(i == 0), stop=(i == KC - 1),
                                )
                            nc.vector.tensor_add(
                                val[:, t16, n * 512:(n + 1) * 512], psum[:],
                                bv_sb[:, n * 512:(n + 1) * 512],
                            )

                # ---------------- per 512-wide sq block ----------------
                for blk in range(NBLK):
                    # queryT block [d, 512] = Wq.T @ qT_blk, scaled 1/32 (+bq/32)
                    Wq_t = load_w(Wq_d)
                    qin = []
                    for i in range(KC):
                        t = inpool.tile([128, 512], BF16, tag="inp", name=f"in{i}")
                        nc.sync.dma_start(
                            out=t[:],
                            in_=qT_d[b, i * 128:(i + 1) * 128, blk * 512:(blk + 1) * 512],
                        )
                        qin.append(t)
                    qry = []
                    for do in range(KC):
                        psum = pspool.tile([128, 512], F32, tag="ps")
                        for i in range(KC):
                            nc.tensor.matmul(
                                psum[:], Wq_t[i][:, do * 128:(do + 1) * 128], qin[i][:],
                                start=(i == 0), stop=(i == KC - 1),
                            )
                        qt = qpool.tile([128, 512], BF16, tag="queryT", name=f"qry{do}")
                        nc.scalar.activation(
                            qt[:], psum[:], AF.Identity,
                            bias=bq_sb[:, do:do + 1], scale=float(SCALE),
                        )
                        qry.append(qt)

                    # scoresT -> expT
                    exp_blk = epool.tile([128, ST, 512], BF16, tag="expT")
                    for t16 in range(ST):
                        psum = pspool.tile([128, 512], F32, tag="ps")
                        for i in range(KC):
                            nc.tensor.matmul(
                                psum[:],
                                keyT[i][:, t16 * 128:(t16 + 1) * 128],
                                qry[i][:],
                                start=(i == 0), stop=(i == KC - 1),
                            )
                        nc.scalar.activation(exp_blk[:, t16, :], psum[:], AF.Exp)

                    # column sums over all sk (partition dim) via ones-matmul
                    sums_ps = ps1pool.tile([1, 512], F32, tag="ps1")
                    for t16 in range(ST):
                        nc.tensor.matmul(
                            sums_ps[:], ones[:], exp_blk[:, t16, :],
                            start=(t16 == 0), stop=(t16 == ST - 1),
                        )
                    sums_sb = smpool.tile([1, 512], F32, tag="sums")
                    nc.scalar.copy(sums_sb[:], sums_ps[:])

                    # UT block [d, 512] = value.T @ expT
                    ut = upool.tile([128, KC, 512], BF16, tag="UT")
                    for j in range(KC):
                        psum = pspool.tile([128, 512], F32, tag="ps")
                        for t16 in range(ST):
                            nc.tensor.matmul(
                                psum[:],
                                val[:, t16, j * 128:(j + 1) * 128],
                                exp_blk[:, t16, :],
                                start=(t16 == 0), stop=(t16 == ST - 1),
                            )
                        nc.scalar.copy(ut[:, j, :], psum[:])

                    # r = 1/sums as per-partition scalars, via [1,128] PE transpose
                    r_sb = smpool.tile([128, 4], F32, tag="r")
                    for m in range(4):
                        pr = psrpool.tile([128, 1], F32, tag="psr")
                        nc.tensor.transpose(
                            pr[:], sums_sb[0:1, m * 128:(m + 1) * 128], ident[:]
                        )
                        nc.vector.reciprocal(r_sb[:, m:m + 1], pr[:])

                    # final block: out[sq, d] = (UT.T @ Wo) * r + bo
                    Wo_t = load_w(Wo_d)
                    for m in range(4):
                        ob = opool.tile([128, D], F32, tag="outb")
                        for n in range(2):
                            psum = pspool.tile([128, 512], F32, tag="ps")
                            for j in range(KC):
                                nc.tensor.matmul(
                                    psum[:],
                                    ut[:, j, m * 128:(m + 1) * 128],
                                    Wo_t[j][:, n * 512:(n + 1) * 512],
                                    start=(j == 0), stop=(j == KC - 1),
                                )
                            nc.vector.tensor_scalar_mul(
                                ob[:, n * 512:(n + 1) * 512], psum[:], r_sb[:, m:m + 1]
                            )
                            nc.vector.tensor_add(
                                ob[:, n * 512:(n + 1) * 512],
                                ob[:, n * 512:(n + 1) * 512],
                                bo_sb[:, n * 512:(n + 1) * 512],
                            )
                        sq = blk * 512 + m * 128
                        nc.sync.dma_start(out=out_d[b, sq:sq + 128, :], in_=ob[:])

    _split_waits(nc)
    return nc


_PROGRAM = None


def _get_program():
    global _PROGRAM
    if _PROGRAM is None:
        _PROGRAM = build_program()
    return _PROGRAM


def prepare_in_maps(q, k, v, Wq, bq, Wk, bk, Wv, bv, Wo, bo):
    bf = ml_dtypes.bfloat16
    f32 = np.float32

    def t_bf16(x):  # [B,S,D] f32 -> [B,D,S] bf16 contiguous
        return np.ascontiguousarray(
            np.asarray(x, f32).astype(bf).transpose(0, 2, 1)
        )

    qT = t_bf16(q)
    kT = t_bf16(k)
    vT = t_bf16(v)
    Wq_b = np.asarray(Wq, f32).astype(bf)
    Wk_b = np.asarray(Wk, f32).astype(bf)
    Wv_b = np.asarray(Wv, f32).astype(bf)
    Wo_b = np.asarray(Wo, f32).astype(bf)
    bq2 = np.ascontiguousarray(
        (np.asarray(bq, f32) * np.float32(SCALE)).reshape(KC, 128).T
    )
    bk2 = np.ascontiguousarray(np.asarray(bk, f32).reshape(KC, 128).T)
    bv1 = np.ascontiguousarray(np.asarray(bv, f32))
    bo1 = np.ascontiguousarray(np.asarray(bo, f32))

    in_maps = []
    for c in range(N_CORES):
        sl = slice(c * NB, (c + 1) * NB)
        in_maps.append({
            "qT": qT[sl], "kT": kT[sl], "vT": vT[sl],
            "Wq": Wq_b, "Wk": Wk_b, "Wv": Wv_b, "Wo": Wo_b,
            "bq": bq2, "bk": bk2, "bv": bv1, "bo": bo1,
        })
    return in_maps


def kernel(q, k, v, Wq, bq, Wk, bk, Wv, bv, Wo, bo):
    nc = _get_program()
    in_maps = prepare_in_maps(q, k, v, Wq, bq, Wk, bk, Wv, bv, Wo, bo)
    res = run_bass_kernel_spmd(nc, in_maps, core_ids=list(range(N_CORES)))
    out = np.concatenate([res.results[c]["out"] for c in range(N_CORES)], axis=0)
    return out.astype(np.float32)
